# revision 50
# baseline (speedup 1.0000x reference)
"""GPT2 attention (B=2, S=2048, E=1024, H=16, interleaved QKV) on 8 trn2 NeuronCores.

Sharding: core c = 4*b + g handles batch b = c//4 and head group g = c%4
(heads 4g..4g+3): Megatron column-split of W_attn / row-split of W_proj,
data-parallel over batch. Host sums the 4 partial projection outputs per batch.

Design (throughput-oriented; baseline 333us -> 213us):
  - X^T is pre-transposed on the host; no PE transposes at all.
  - qk^T = W^T X^T (features on partitions); V computed directly in
    [token, dim] layout via x-stationary matmuls (no V transpose).
  - Scores S^T[sk,sq] per head with 64-deep contraction run 2-way
    concurrent on the two PE row-tiles (tile_position (0,0)/(64,0)),
    one head per half-array; both heads share one [128,1024] score tile
    so every softmax exp call covers two blocks. Diagonal score blocks
    only compute their valid (lower-trapezoid) width.
  - softmax exp is split between ACT (native Exp) and DVE (Schraudolph
    exponent-stuffing: round(x*a+b) as uint16, bitcast to f16) via a
    run-bounded load balancer. Large diagonal blocks (r=0,1) use exact
    ACT exp + a DVE 0/1 mask multiply; small ones (r=2,3) use a single
    fused DVE op whose mask-bias operand drives masked entries negative
    so the u16 convert saturates them to exactly 0.
  - PV appends a ones column to V so PSUM row 64 = softmax denominators;
    1/den via an f16 bit-trick seed + one Newton step on the denominator
    rows, broadcast with a rank-1 PE matmul kept in the same (64,128)
    tile mode as the score stream, then one DVE multiply. The broadcast
    matmuls are deferred into the next group's score stream so their DVE
    inputs are ready when the PE reaches them.
  - the output projection is interleaved into the attention stream (one
    sq-chunk right after each PV burst, same 128x128 tile mode), its
    PSUM shared with the broadcast pool; evacuation is f16 alternating
    ACT/DVE, halving the output DMA.
  - input DMAs are spread over the sync/scalar/gpsimd queues (each
    dma_start costs ~650ns of queue issue time) with wa[0]/xT[0] first.
"""
import numpy as np

import concourse.bass as bass
import concourse.bacc as bacc
import concourse.tile as tile
from concourse import mybir
from concourse.bass_utils import run_bass_kernel_spmd

F32 = mybir.dt.float32
F16 = mybir.dt.float16
I16 = mybir.dt.int16
U16 = mybir.dt.uint16

B, S, E, H = 2, 2048, 1024, 16
HD = E // H            # 64
HPC = 4                # heads per core
CW = HPC * 3 * HD      # 768: W_attn cols per core
CP = HPC * HD          # 256: W_proj rows per core
NK = E // 128          # 8 contraction chunks over E
NSQ = S // 512         # 4 sq chunks of 512
NSK = S // 128         # 16 sk chunks of 128

# Schraudolph exp: exp(x) ~= bitcast_f16(round(x * SCH_A + SCH_B)); the u16
# output convert saturates negatives to 0, so masked entries (biased by
# -60000 via the fused mask operand) become exactly +0.0
SCH_A = 1024.0 / float(np.log(2.0))
SCH_B = 15312.0  # 15360 - 48, f16-exact so the mask constant tiles match
MASKED = -60000.0
# f16 reciprocal seed: 1/d ~= bitcast_f16(RCP_K - bits_f16(d)), then one
# Newton step y1 = y0 * (2 - d*y0); max rel err ~3e-3
RCP_K = 30620.0

# dispatch cost model (ns) for the exp of a [128, n]-column pair tile
ACT_COST = lambda n: (n + 352) / 1.2
DVE_COST = lambda n: 0.52 * n + 300.0
DVE_NORM_EXTRA = 5000.0  # den copies + newton + bcs + muls per (pair, J)

_cache = {}
_last_in_maps = None


def _build():
    from contextlib import ExitStack

    nc = bacc.Bacc("TRN2", target_bir_lowering=False, debug=False, num_devices=8)

    x_d = nc.dram_tensor("x", [NK, 128, S], F16, kind="ExternalInput").ap()
    wa_d = nc.dram_tensor("wa", [NK, 128, CW], F16, kind="ExternalInput").ap()
    ba_d = nc.dram_tensor("ba", [1, 128, 4], F32, kind="ExternalInput").ap()
    bv_d = nc.dram_tensor("bv", [1, 1, CP], F16, kind="ExternalInput").ap()
    wp_d = nc.dram_tensor("wp", [2, 128, E], F16, kind="ExternalInput").ap()
    out_d = nc.dram_tensor("out_t", [8, 128, S], F16, kind="ExternalOutput").ap()

    # diagonal-block masks in pair layout [h0: 0..w | gap | h1: 512..512+w |
    # gap], reduced coords g (block col), keep where g >= p.
    # r=0,1 (large blocks): 0/1 multiply masks applied after an exact ACT exp.
    # r=2,3 (small blocks): fused bias masks for the DVE Schraudolph exp
    # (keep -> +SCH_B, masked -> -60000 so the u16 convert saturates to 0).
    gi = np.arange(512)[None, :]
    pi = np.arange(128)[:, None]
    mask01_d = {}
    maskB_d = {}
    for r in range(4):
        w = 512 - 128 * r
        keep = (gi < w) & (gi >= pi)
        if r < 2:
            half = np.zeros((128, 512), np.float16)
            half[keep] = 1.0
            mask01_d[r] = nc.inline_tensor(
                np.concatenate([half, half], axis=1), name=f"mask01_{r}"
            )
        else:
            half = np.full((128, 512), MASKED, np.float16)
            half[keep] = SCH_B
            maskB_d[r] = nc.inline_tensor(
                np.concatenate([half, half], axis=1), name=f"maskB{r}"
            )
    ones1_d = nc.inline_tensor(np.ones((1, 128), np.float16), name="ones1")
    # rank-1 broadcast stationaries: row 64 (and 65) = ones, used in the
    # (64,128) tile mode so they do not break the score-stream mode
    oneblk_np = np.zeros((128, 128), np.float16)
    oneblk_np[64, :] = 1.0
    oneblk96_np = np.zeros((128, 128), np.float16)
    oneblk96_np[96, :] = 1.0
    oneblk_d = nc.inline_tensor(oneblk_np, name="oneblk")
    oneblk96_d = nc.inline_tensor(oneblk96_np, name="oneblk96")

    Exp = mybir.ActivationFunctionType.Exp
    Ident = mybir.ActivationFunctionType.Identity
    Mult = mybir.AluOpType.mult
    Add = mybir.AluOpType.add

    with tile.TileContext(nc) as tc, ExitStack() as top:
        consts = top.enter_context(tc.tile_pool(name="consts", bufs=1))
        qk_pool = top.enter_context(tc.tile_pool(name="qkT", bufs=1))
        at_pool = top.enter_context(tc.tile_pool(name="attnT", bufs=1))
        wp_pool = top.enter_context(tc.tile_pool(name="wp", bufs=1))
        vb_pool = top.enter_context(tc.tile_pool(name="vb", bufs=1))
        xTp = top.enter_context(tc.tile_pool(name="xT", bufs=1))

        # ---- input DMAs -------------------------------------------------
        # each dma_start costs ~650ns of queue issue time, so spread the
        # loads over four queues and put the critical first inputs (wa[0],
        # xT[0]) at the head of their queues
        xT = [xTp.tile([128, S], F16, tag=f"xT{k}", name=f"xT{k}") for k in range(NK)]
        wa_t = consts.tile([128, NK, CW], F16)
        dq = [nc.sync, nc.scalar, nc.gpsimd]
        for k in range(NK):
            nc.gpsimd.dma_start(out=wa_t[:, k, :], in_=wa_d[k])
            dq[k % 3].dma_start(out=xT[k][:], in_=x_d[k])
        ba_t = consts.tile([128, 4], F32)
        nc.scalar.dma_start(out=ba_t[:], in_=ba_d[0])
        ones1_t = consts.tile([1, 128], F16)
        nc.scalar.dma_start(out=ones1_t[:], in_=ones1_d.ap())
        bv_t = consts.tile([1, CP], F16)
        nc.scalar.dma_start(out=bv_t[:], in_=bv_d[0])
        wp_t = wp_pool.tile([128, 2, E], F16)
        for cc in range(2):
            nc.sync.dma_start(out=wp_t[:, cc, :], in_=wp_d[cc])
        mask01_t = consts.tile([128, 2, 1024], F16)
        for r in range(2):
            nc.sync.dma_start(out=mask01_t[:, r, :], in_=mask01_d[r].ap())
        maskB_t = consts.tile([128, 2, 1024], F16)
        for r in range(2, 4):
            nc.sync.dma_start(out=maskB_t[:, r - 2, :], in_=maskB_d[r].ap())

        qkvT = [
            qk_pool.tile([128, S], F16, tag=f"qkT{cc}", name=f"qkT{cc}")
            for cc in range(4)
        ]
        attnT = [
            at_pool.tile([128, S], F16, tag=f"attnT{c}", name=f"attnT{c}")
            for c in range(2)
        ]
        # V with a ones column appended: [sk-chunk partitions, i, head, 65]
        vb4 = vb_pool.tile([128, NSK, HPC, 65], F16)
        nc.gpsimd.memset(vb4[:, :, :, 64:65], 1.0)

        # ---- phase 1: qk^T = W^T X^T, V = X Wv --------------------------
        with (
            tc.tile_pool(name="ps_mm", bufs=4, space="PSUM") as ps_mm,
            tc.tile_pool(name="ps_v", bufs=3, space="PSUM") as ps_v,
            tc.tile_pool(name="ps_b", bufs=1, space="PSUM") as ps_b,
        ):
            bvbc = consts.tile([128, HPC, 64], F32)

            for cc in range(4):
                if cc == 1:
                    # v-bias broadcast (32-row tile mode), tucked in after
                    # cc=0 so the kernel's first matmul is not gated on the
                    # small-constant DMAs
                    psb = ps_b.tile([128, HPC, 64], F32, tag="pvb", name="pvb")
                    nc.tensor.matmul(
                        psb[:], ones1_t[0:1, :], bv_t[0:1, :], start=True, stop=True
                    )
                    nc.vector.tensor_copy(bvbc[:], psb[:])
                pss = [
                    ps_mm.tile([128, 512], F32, tag="mm", name="mm_ps")
                    for _ in range(4)
                ]
                for k in range(NK):
                    lhs = wa_t[:, k, cc * 128 : (cc + 1) * 128]
                    for rc in range(4):
                        nc.tensor.matmul(
                            pss[rc][:],
                            lhs,
                            xT[k][:, rc * 512 : (rc + 1) * 512],
                            start=(k == 0),
                            stop=(k == NK - 1),
                        )
                for rc in range(4):
                    nc.scalar.activation(
                        qkvT[cc][:, rc * 512 : (rc + 1) * 512],
                        pss[rc][:],
                        Ident,
                        bias=ba_t[:, cc : cc + 1],
                        scale=0.125 if cc < 2 else 1.0,
                    )

            for i in range(NSK):
                psv = ps_v.tile([128, HPC, 64], F32, tag="pv", name="pv_ps")
                for k in range(NK):
                    nc.tensor.matmul(
                        psv[:],
                        xT[k][:, i * 128 : (i + 1) * 128],
                        wa_t[:, k, 512:768],
                        start=(k == 0),
                        stop=(k == NK - 1),
                    )
                nc.vector.tensor_add(vb4[:, i, :, 0:64], psv[:], bvbc[:])

        # ---- phase 2: per-head-pair attention ---------------------------
        # Both heads of a pair share one [128,1024] score tile (one bank per
        # head), so every exp call covers two blocks. Diagonal blocks use the
        # DVE fused op (x*A + maskB) with u16 saturation; off-diagonal blocks
        # are load-balanced between ACT Exp and DVE Schraudolph.
        with (
            tc.tile_pool(name="pp", bufs=18) as p_pool,
            tc.tile_pool(name="sm", bufs=4) as small,
            tc.tile_pool(name="rcf", bufs=1) as rcf_pool,
            tc.tile_pool(name="ps_s", bufs=2, space="PSUM") as ps_s,
            tc.tile_pool(name="ps_pv", bufs=1, space="PSUM") as ps_pv,
            tc.tile_pool(name="ps_bc", bufs=2, space="PSUM") as ps_bc,
            tc.tile_pool(name="ob", bufs=6) as ob_pool,
        ):
            oneblk_t = consts.tile([128, 128], F16)
            nc.gpsimd.dma_start(out=oneblk_t[:], in_=oneblk_d.ap())
            oneblk96_t = consts.tile([128, 128], F16)
            nc.gpsimd.dma_start(out=oneblk96_t[:], in_=oneblk96_d.ap())
            # rcp rows ring: rows 64/96 carry 1/den (h0/h1) as f16 for the
            # rank-1 broadcast matmuls (engine partition starts must be
            # 32-aligned); other rows zeroed once (NaN guard)
            # guard rows start at 1.0: the newton ops iterate y <- ~1/y on
            # them, for which 1.0 is a stable bounded fixed point (0.0 would
            # blow up to inf and 0*inf would NaN the broadcast matmul)
            rcf_ring = []
            for n in range(4):
                t = rcf_pool.tile([128, 512], F16, tag=f"rcf{n}", name=f"rcf{n}")
                nc.vector.memset(t[64:128, :], 1.0)
                rcf_ring.append(t)

            act_t = 0.0
            dve_t = 0.0
            last_eng = []
            pending = []
            nidx = 0

            def emit_proj(Jq):
                # projection for sq chunk Jq, interleaved into the attention
                # stream right after a PV burst (same 128x128 tile mode).
                # Alternates between the ps_bc banks and (idle at this point)
                # ps_s banks so four accumulators rotate and the matmuls do
                # not serialize behind the evacuation copies.
                for eo in range(8):
                    if eo % 2 == 0:
                        ppt = ps_bc.tile([128, 512], F32, tag="bc", name="prj")
                    else:
                        ppt = ps_s.tile([128, 1024], F32, tag="s", name="prj")
                    pp = ppt[:, 0:512]
                    for cc in range(2):
                        nc.tensor.matmul(
                            pp,
                            wp_t[:, cc, eo * 128 : (eo + 1) * 128],
                            attnT[cc][:, Jq * 512 : (Jq + 1) * 512],
                            start=(cc == 0),
                            stop=(cc == 1),
                        )
                    ob = ob_pool.tile([128, 512], F16, tag="ob", name="ob")
                    if eo % 2 == 0:
                        nc.scalar.copy(ob[:], pp)
                    else:
                        nc.vector.tensor_copy(ob[:], pp)
                    eng = nc.sync if eo % 2 == 0 else nc.scalar
                    eng.dma_start(
                        out=out_d[eo][:, Jq * 512 : (Jq + 1) * 512], in_=ob[:]
                    )

            for J in range(NSQ):
                for pr in range(2):
                    qT = qkvT[pr]
                    kT = qkvT[2 + pr]
                    nblk = 4 * J + 4
                    sq = bass.ts(J, 512)
                    # block order: diagonal r=0..3 first, then off-diagonal
                    order = [4 * J + r for r in range(4)] + list(range(4 * J))
                    pblks = {}
                    nissued = 0
                    for i in order:
                        r = i - 4 * J
                        w = 512 if r < 0 else 512 - 128 * r
                        sqo = J * 512 + (0 if r < 0 else 128 * r)
                        stile = ps_s.tile([128, 1024], F32, tag="s", name="sps")
                        for hh in range(2):
                            o = hh * 64
                            nc.tensor.matmul(
                                stile[:, hh * 512 : hh * 512 + w],
                                kT[o : o + 64, i * 128 : (i + 1) * 128],
                                qT[o : o + 64, sqo : sqo + w],
                                start=True,
                                stop=True,
                                tile_position=(o, 0),
                            )
                        pt = p_pool.tile([128, 1024], U16, tag="p", name="p")
                        pf = pt[:].bitcast(F16)
                        if r >= 0 and r < 2:
                            # large diagonal blocks: exact exp on ACT, then a
                            # fast 0/1 mask multiply on DVE
                            act_t += ACT_COST(1024)
                            nc.scalar.activation(pf, stile[:], Exp)
                            dve_t += 600.0
                            nc.vector.tensor_mul(pf, pf, mask01_t[:, r, :])
                        elif r >= 2:
                            # small diagonal blocks: one fused DVE op
                            # (Schraudolph exp + mask bias, u16 saturation)
                            dve_t += DVE_COST(1024)
                            nc.vector.scalar_tensor_tensor(
                                out=pt[:],
                                in0=stile[:],
                                scalar=SCH_A,
                                in1=maskB_t[:, r - 2, :],
                                op0=Mult,
                                op1=Add,
                            )
                        else:
                            ca, cd = ACT_COST(1024), DVE_COST(1024)
                            use_act = act_t + ca <= dve_t + cd
                            # bound same-engine runs at 2 so the two
                            # in-flight score tiles are never serialized
                            # behind a single engine
                            if len(last_eng) >= 2 and last_eng[-1] == last_eng[-2]:
                                use_act = not last_eng[-1]
                            last_eng.append(use_act)
                            if use_act:
                                act_t += ca
                                nc.scalar.activation(pf, stile[:], Exp)
                            else:
                                dve_t += cd
                                nc.vector.tensor_scalar(
                                    pt[:], stile[:], SCH_A, SCH_B, Mult, Add
                                )
                        pblks[i] = pt
                        nissued += 1
                        # the previous group's broadcast matmuls + final muls
                        # land here: same (64,128) PE tile mode as the S
                        # stream, and their DVE inputs are ready by now
                        if nissued == 3 and pending:
                            for fn in pending:
                                fn()
                            pending = []
                    if pending:
                        for fn in pending:
                            fn()
                        pending = []

                    rcf = rcf_ring[nidx % 4]
                    nidx += 1
                    pvp = ps_pv.tile([65, 1024], F32, tag="pv", name="pvp")
                    for hh in range(2):
                        for n, i in enumerate(order):
                            r = i - 4 * J
                            w = 512 if r < 0 else 512 - 128 * r
                            co = hh * 512 + (0 if r < 0 else 128 * r)
                            nc.tensor.matmul(
                                pvp[:, co : co + w],
                                vb4[:, i, 2 * pr + hh, :],
                                pblks[i][:, hh * 512 : hh * 512 + w].bitcast(F16),
                                start=(n == 0),
                                stop=(n == nblk - 1),
                            )
                        if hh == 0:
                            # h0's denominator row is final: stage its copy
                            # so it overlaps the h1 PV burst on the DVE
                            nc.vector.tensor_copy(
                                rcf[64:65, :], pvp[64:65, 0:512]
                            )

                    # 1/den for both heads: f16 bit-trick seed + one Newton
                    # step on rcf rows 64 (h0) and 96 (h1); the ops run over
                    # the whole [64:128] row block (32-aligned starts) — the
                    # other rows turn into finite garbage that the zero rows
                    # of the broadcast stationary annihilate
                    nc.vector.tensor_copy(rcf[96:97, :], pvp[64:65, 512:1024])
                    y0 = small.tile([128, 512], I16, tag="y0", name="y0")
                    nc.vector.tensor_scalar(
                        y0[64:128, :], rcf[64:128, :].bitcast(I16),
                        -1.0, RCP_K, Mult, Add,
                    )
                    t1 = small.tile([128, 512], F16, tag="t1", name="t1")
                    nc.vector.tensor_mul(
                        t1[64:128, :], rcf[64:128, :], y0[64:128, :].bitcast(F16)
                    )
                    nc.vector.tensor_scalar(
                        t1[64:128, :], t1[64:128, :], -1.0, 2.0, Mult, Add
                    )
                    nc.vector.tensor_mul(
                        rcf[64:128, :], y0[64:128, :].bitcast(F16), t1[64:128, :]
                    )
                    dve_t += DVE_NORM_EXTRA

                    def norm(pvp=pvp, sq=sq, pr=pr, rcf=rcf):
                        for hh, blk in ((0, oneblk_t), (1, oneblk96_t)):
                            o = hh * 64
                            bcp = ps_bc.tile(
                                [128, 512], F32, tag="bc", name="bcp"
                            )
                            nc.tensor.matmul(
                                bcp[:],
                                blk[64:128, :],
                                rcf[64:128, :],
                                start=True,
                                stop=True,
                                tile_position=(64, 0),
                            )
                            bcs = small.tile(
                                [64, 512], F32, tag="bcs", name="bcs"
                            )
                            nc.vector.tensor_copy(bcs[:], bcp[0:64, :])
                            nc.vector.tensor_mul(
                                attnT[pr][o : o + 64, sq],
                                pvp[0:64, hh * 512 : (hh + 1) * 512],
                                bcs[:],
                            )

                    pending.append(norm)
                    if pr == 0 and J >= 1:
                        emit_proj(J - 1)
            for fn in pending:
                fn()
            emit_proj(3)

    nc.compile()
    return nc


def _col_perm(g):
    """Per-core W_attn column permutation: [q0..q3 | k0..k3 | v0..v3]."""
    cols = []
    for t in range(3):          # q, k, v
        for h in range(HPC):
            base = (4 * g + h) * 3 * HD + t * HD
            cols.append(np.arange(base, base + HD))
    return np.concatenate(cols)


def kernel(hidden_states, W_attn, b_attn, W_proj, b_proj):
    hidden_states = np.asarray(hidden_states, np.float32)
    W_attn = np.asarray(W_attn, np.float32)
    b_attn = np.asarray(b_attn, np.float32)
    W_proj = np.asarray(W_proj, np.float32)
    b_proj = np.asarray(b_proj, np.float32)

    if "nc" not in _cache:
        _cache["nc"] = _build()
    nc = _cache["nc"]

    # q columns (first 256 of the permuted layout) have scale 1/8 folded into
    # the PSUM->SBUF copy; bias is added after the scale, so pre-scale it.
    bias_scale = np.ones(2 * CP, np.float32)
    bias_scale[:CP] = 0.125

    in_maps = []
    for c in range(8):
        b, g = divmod(c, 4)
        perm = _col_perm(g)
        wa = np.ascontiguousarray(W_attn[:, perm])
        ba = np.ascontiguousarray(
            (b_attn[perm][: 2 * CP] * bias_scale).astype(np.float32).reshape(4, 128).T
        )
        bv = b_attn[perm][2 * CP :].astype(np.float16)
        wp = np.ascontiguousarray(W_proj[g * CP : (g + 1) * CP, :])
        xT = np.ascontiguousarray(hidden_states[b].T).astype(np.float16)
        in_maps.append(
            {
                "x": xT.reshape(NK, 128, S),
                "wa": wa.astype(np.float16).reshape(NK, 128, CW),
                "ba": ba.reshape(1, 128, 4),
                "bv": bv.reshape(1, 1, CP),
                "wp": wp.astype(np.float16).reshape(2, 128, E),
            }
        )

    global _last_in_maps
    _last_in_maps = in_maps
    res = run_bass_kernel_spmd(nc, in_maps, list(range(8)))

    out = np.zeros((B, S, E), np.float32)
    for c in range(8):
        b = c // 4
        out[b] += res.results[c]["out_t"].reshape(E, S).astype(np.float32).T
    out += b_proj
    return out


# revision 53
# speedup vs baseline: 1.0274x; 1.0274x over previous
"""GPT2 attention (B=2, S=2048, E=1024, H=16, interleaved QKV) on 8 trn2 NeuronCores.

Sharding: core c = 4*b + g handles batch b = c//4 and head group g = c%4
(heads 4g..4g+3): Megatron column-split of W_attn / row-split of W_proj,
data-parallel over batch. Host sums the 4 partial projection outputs per batch.

Design (throughput-oriented; baseline 333us -> 213us):
  - X^T is pre-transposed on the host; no PE transposes at all.
  - qk^T = W^T X^T (features on partitions); V computed directly in
    [token, dim] layout via x-stationary matmuls (no V transpose).
  - Scores S^T[sk,sq] per head with 64-deep contraction run 2-way
    concurrent on the two PE row-tiles (tile_position (0,0)/(64,0)),
    one head per half-array; both heads share one [128,1024] score tile
    so every softmax exp call covers two blocks. Diagonal score blocks
    only compute their valid (lower-trapezoid) width.
  - softmax exp is split between ACT (native Exp) and DVE (Schraudolph
    exponent-stuffing: round(x*a+b) as uint16, bitcast to f16) via a
    run-bounded load balancer. Large diagonal blocks (r=0,1) use exact
    ACT exp + a DVE 0/1 mask multiply; small ones (r=2,3) use a single
    fused DVE op whose mask-bias operand drives masked entries negative
    so the u16 convert saturates them to exactly 0.
  - PV appends a ones column to V so PSUM row 64 = softmax denominators;
    1/den via an f16 bit-trick seed + one Newton step on the denominator
    rows, broadcast with a rank-1 PE matmul kept in the same (64,128)
    tile mode as the score stream, then one DVE multiply. The broadcast
    matmuls are deferred into the next group's score stream so their DVE
    inputs are ready when the PE reaches them.
  - the output projection is interleaved into the attention stream (one
    sq-chunk right after each PV burst, same 128x128 tile mode), its
    PSUM shared with the broadcast pool; evacuation is f16 alternating
    ACT/DVE, halving the output DMA.
  - input DMAs are spread over the sync/scalar/gpsimd queues (each
    dma_start costs ~650ns of queue issue time) with wa[0]/xT[0] first.
"""
import numpy as np

import concourse.bass as bass
import concourse.bacc as bacc
import concourse.tile as tile
from concourse import mybir
from concourse.bass_utils import run_bass_kernel_spmd

F32 = mybir.dt.float32
F16 = mybir.dt.float16
I16 = mybir.dt.int16
U16 = mybir.dt.uint16

B, S, E, H = 2, 2048, 1024, 16
HD = E // H            # 64
HPC = 4                # heads per core
CW = HPC * 3 * HD      # 768: W_attn cols per core
CP = HPC * HD          # 256: W_proj rows per core
NK = E // 128          # 8 contraction chunks over E
NSQ = S // 512         # 4 sq chunks of 512
NSK = S // 128         # 16 sk chunks of 128

# Schraudolph exp: exp(x) ~= bitcast_f16(round(x * SCH_A + SCH_B)); the u16
# output convert saturates negatives to 0, so masked entries (biased by
# -60000 via the fused mask operand) become exactly +0.0
SCH_A = 1024.0 / float(np.log(2.0))
SCH_B = 15312.0  # 15360 - 48, f16-exact so the mask constant tiles match
MASKED = -60000.0
# f16 reciprocal seed: 1/d ~= bitcast_f16(RCP_K - bits_f16(d)), then one
# Newton step y1 = y0 * (2 - d*y0); max rel err ~3e-3
RCP_K = 30620.0

# dispatch cost model (ns) for the exp of a [128, n]-column pair tile
ACT_COST = lambda n: (n + 352) / 1.2
DVE_COST = lambda n: 0.52 * n + 300.0
DVE_NORM_EXTRA = 5000.0  # den copies + newton + bcs + muls per (pair, J)

_cache = {}
_last_in_maps = None


def _build():
    from contextlib import ExitStack

    nc = bacc.Bacc("TRN2", target_bir_lowering=False, debug=False, num_devices=8)

    x_d = nc.dram_tensor("x", [NK, 128, S], F16, kind="ExternalInput").ap()
    wa_d = nc.dram_tensor("wa", [NK, 128, CW], F16, kind="ExternalInput").ap()
    ba_d = nc.dram_tensor("ba", [1, 128, 4], F32, kind="ExternalInput").ap()
    bv_d = nc.dram_tensor("bv", [1, 1, CP], F16, kind="ExternalInput").ap()
    wp_d = nc.dram_tensor("wp", [2, 128, E], F16, kind="ExternalInput").ap()
    out_d = nc.dram_tensor("out_t", [8, 128, S], F16, kind="ExternalOutput").ap()

    # diagonal-block masks in pair layout [h0: 0..w | gap | h1: 512..512+w |
    # gap], reduced coords g (block col), keep where g >= p.
    # r=0,1 (large blocks): 0/1 multiply masks applied after an exact ACT exp.
    # r=2,3 (small blocks): fused bias masks for the DVE Schraudolph exp
    # (keep -> +SCH_B, masked -> -60000 so the u16 convert saturates to 0).
    gi = np.arange(512)[None, :]
    pi = np.arange(128)[:, None]
    mask01_d = {}
    maskB_d = {}
    for r in range(4):
        w = 512 - 128 * r
        keep = (gi < w) & (gi >= pi)
        if r < 2:
            half = np.zeros((128, 512), np.float16)
            half[keep] = 1.0
            mask01_d[r] = nc.inline_tensor(
                np.concatenate([half, half], axis=1), name=f"mask01_{r}"
            )
        else:
            half = np.full((128, 512), MASKED, np.float16)
            half[keep] = SCH_B
            maskB_d[r] = nc.inline_tensor(
                np.concatenate([half, half], axis=1), name=f"maskB{r}"
            )
    ones1_d = nc.inline_tensor(np.ones((1, 128), np.float16), name="ones1")
    # rank-1 broadcast stationaries: row 64 (and 65) = ones, used in the
    # (64,128) tile mode so they do not break the score-stream mode
    oneblk_np = np.zeros((128, 128), np.float16)
    oneblk_np[64, :] = 1.0
    oneblk96_np = np.zeros((128, 128), np.float16)
    oneblk96_np[96, :] = 1.0
    oneblk_d = nc.inline_tensor(oneblk_np, name="oneblk")
    oneblk96_d = nc.inline_tensor(oneblk96_np, name="oneblk96")

    Exp = mybir.ActivationFunctionType.Exp
    Ident = mybir.ActivationFunctionType.Identity
    Mult = mybir.AluOpType.mult
    Add = mybir.AluOpType.add

    with tile.TileContext(nc) as tc, ExitStack() as top:
        consts = top.enter_context(tc.tile_pool(name="consts", bufs=1))
        qk_pool = top.enter_context(tc.tile_pool(name="qkT", bufs=1))
        at_pool = top.enter_context(tc.tile_pool(name="attnT", bufs=1))
        wp_pool = top.enter_context(tc.tile_pool(name="wp", bufs=1))
        vb_pool = top.enter_context(tc.tile_pool(name="vb", bufs=1))
        xTp = top.enter_context(tc.tile_pool(name="xT", bufs=1))

        # ---- input DMAs -------------------------------------------------
        # each dma_start costs ~650ns of queue issue time, so spread the
        # loads over four queues and put the critical first inputs (wa[0],
        # xT[0]) at the head of their queues
        xT = [xTp.tile([128, S], F16, tag=f"xT{k}", name=f"xT{k}") for k in range(NK)]
        wa_t = consts.tile([128, NK, CW], F16)
        dq = [nc.sync, nc.scalar]
        for k in range(NK):
            nc.gpsimd.dma_start(out=wa_t[:, k, :], in_=wa_d[k])
            dq[k % 2].dma_start(out=xT[k][:], in_=x_d[k])
        ba_t = consts.tile([128, 4], F32)
        nc.scalar.dma_start(out=ba_t[:], in_=ba_d[0])
        ones1_t = consts.tile([1, 128], F16)
        nc.scalar.dma_start(out=ones1_t[:], in_=ones1_d.ap())
        bv_t = consts.tile([1, CP], F16)
        nc.scalar.dma_start(out=bv_t[:], in_=bv_d[0])
        wp_t = wp_pool.tile([128, 2, E], F16)
        for cc in range(2):
            nc.sync.dma_start(out=wp_t[:, cc, :], in_=wp_d[cc])
        mask01_t = consts.tile([128, 2, 1024], F16)
        for r in range(2):
            nc.sync.dma_start(out=mask01_t[:, r, :], in_=mask01_d[r].ap())
        maskB_t = consts.tile([128, 2, 1024], F16)
        for r in range(2, 4):
            nc.sync.dma_start(out=maskB_t[:, r - 2, :], in_=maskB_d[r].ap())

        qkvT = [
            qk_pool.tile([128, S], F16, tag=f"qkT{cc}", name=f"qkT{cc}")
            for cc in range(4)
        ]
        attnT = [
            at_pool.tile([128, S], F16, tag=f"attnT{c}", name=f"attnT{c}")
            for c in range(2)
        ]
        # V with a ones column appended: [sk-chunk partitions, i, head, 65]
        vb4 = vb_pool.tile([128, NSK, HPC, 65], F16)
        nc.gpsimd.memset(vb4[:, :, :, 64:65], 1.0)

        # ---- phase 1: qk^T = W^T X^T, V = X Wv --------------------------
        with (
            tc.tile_pool(name="ps_mm", bufs=4, space="PSUM") as ps_mm,
            tc.tile_pool(name="ps_v", bufs=3, space="PSUM") as ps_v,
            tc.tile_pool(name="ps_b", bufs=1, space="PSUM") as ps_b,
        ):
            bvbc = consts.tile([128, HPC, 64], F32)

            for cc in range(4):
                if cc == 1:
                    # v-bias broadcast (32-row tile mode), tucked in after
                    # cc=0 so the kernel's first matmul is not gated on the
                    # small-constant DMAs
                    psb = ps_b.tile([128, HPC, 64], F32, tag="pvb", name="pvb")
                    nc.tensor.matmul(
                        psb[:], ones1_t[0:1, :], bv_t[0:1, :], start=True, stop=True
                    )
                    nc.vector.tensor_copy(bvbc[:], psb[:])
                pss = [
                    ps_mm.tile([128, 512], F32, tag="mm", name="mm_ps")
                    for _ in range(4)
                ]
                for k in range(NK):
                    lhs = wa_t[:, k, cc * 128 : (cc + 1) * 128]
                    for rc in range(4):
                        nc.tensor.matmul(
                            pss[rc][:],
                            lhs,
                            xT[k][:, rc * 512 : (rc + 1) * 512],
                            start=(k == 0),
                            stop=(k == NK - 1),
                        )
                for rc in range(4):
                    nc.scalar.activation(
                        qkvT[cc][:, rc * 512 : (rc + 1) * 512],
                        pss[rc][:],
                        Ident,
                        bias=ba_t[:, cc : cc + 1],
                        scale=0.125 if cc < 2 else 1.0,
                    )

            for i in range(NSK):
                psv = ps_v.tile([128, HPC, 64], F32, tag="pv", name="pv_ps")
                for k in range(NK):
                    nc.tensor.matmul(
                        psv[:],
                        xT[k][:, i * 128 : (i + 1) * 128],
                        wa_t[:, k, 512:768],
                        start=(k == 0),
                        stop=(k == NK - 1),
                    )
                nc.vector.tensor_add(vb4[:, i, :, 0:64], psv[:], bvbc[:])

        # ---- phase 2: per-head-pair attention ---------------------------
        # Both heads of a pair share one [128,1024] score tile (one bank per
        # head), so every exp call covers two blocks. Diagonal blocks use the
        # DVE fused op (x*A + maskB) with u16 saturation; off-diagonal blocks
        # are load-balanced between ACT Exp and DVE Schraudolph.
        with (
            tc.tile_pool(name="pp", bufs=18) as p_pool,
            tc.tile_pool(name="sm", bufs=4) as small,
            tc.tile_pool(name="rcf", bufs=1) as rcf_pool,
            tc.tile_pool(name="ps_s", bufs=2, space="PSUM") as ps_s,
            tc.tile_pool(name="ps_pv", bufs=1, space="PSUM") as ps_pv,
            tc.tile_pool(name="ps_bc", bufs=2, space="PSUM") as ps_bc,
            tc.tile_pool(name="ob", bufs=6) as ob_pool,
        ):
            oneblk_t = consts.tile([128, 128], F16)
            nc.gpsimd.dma_start(out=oneblk_t[:], in_=oneblk_d.ap())
            oneblk96_t = consts.tile([128, 128], F16)
            nc.gpsimd.dma_start(out=oneblk96_t[:], in_=oneblk96_d.ap())
            # rcp rows ring: rows 64/96 carry 1/den (h0/h1) as f16 for the
            # rank-1 broadcast matmuls (engine partition starts must be
            # 32-aligned); other rows zeroed once (NaN guard)
            # guard rows start at 1.0: the newton ops iterate y <- ~1/y on
            # them, for which 1.0 is a stable bounded fixed point (0.0 would
            # blow up to inf and 0*inf would NaN the broadcast matmul)
            rcf_ring = []
            for n in range(4):
                t = rcf_pool.tile([128, 512], F16, tag=f"rcf{n}", name=f"rcf{n}")
                nc.vector.memset(t[64:128, :], 1.0)
                rcf_ring.append(t)

            act_t = 0.0
            dve_t = 0.0
            last_eng = []
            pending = []
            nidx = 0

            def emit_proj(Jq):
                # projection for sq chunk Jq, interleaved into the attention
                # stream right after a PV burst (same 128x128 tile mode);
                # shares the ps_bc pool's banks
                for eo in range(8):
                    pp = ps_bc.tile([128, 512], F32, tag="bc", name="prj")
                    for cc in range(2):
                        nc.tensor.matmul(
                            pp[:],
                            wp_t[:, cc, eo * 128 : (eo + 1) * 128],
                            attnT[cc][:, Jq * 512 : (Jq + 1) * 512],
                            start=(cc == 0),
                            stop=(cc == 1),
                        )
                    ob = ob_pool.tile([128, 512], F16, tag="ob", name="ob")
                    if eo % 2 == 0:
                        nc.scalar.copy(ob[:], pp[:])
                    else:
                        nc.vector.tensor_copy(ob[:], pp[:])
                    # all output triggers on the (otherwise idle) sync queue:
                    # a trigger + its semaphore wait on the ACT queue blocks
                    # the strict FIFO ahead of pending exp work
                    nc.sync.dma_start(
                        out=out_d[eo][:, Jq * 512 : (Jq + 1) * 512], in_=ob[:]
                    )

            for J in range(NSQ):
                for pr in range(2):
                    qT = qkvT[pr]
                    kT = qkvT[2 + pr]
                    nblk = 4 * J + 4
                    sq = bass.ts(J, 512)
                    # block order: diagonal r=0..3 first, then off-diagonal
                    order = [4 * J + r for r in range(4)] + list(range(4 * J))
                    pblks = {}
                    nissued = 0
                    for i in order:
                        r = i - 4 * J
                        w = 512 if r < 0 else 512 - 128 * r
                        sqo = J * 512 + (0 if r < 0 else 128 * r)
                        stile = ps_s.tile([128, 1024], F32, tag="s", name="sps")
                        for hh in range(2):
                            o = hh * 64
                            nc.tensor.matmul(
                                stile[:, hh * 512 : hh * 512 + w],
                                kT[o : o + 64, i * 128 : (i + 1) * 128],
                                qT[o : o + 64, sqo : sqo + w],
                                start=True,
                                stop=True,
                                tile_position=(o, 0),
                            )
                        pt = p_pool.tile([128, 1024], U16, tag="p", name="p")
                        pf = pt[:].bitcast(F16)
                        if r >= 0 and r < 2:
                            # large diagonal blocks: exact exp on ACT, then a
                            # fast 0/1 mask multiply on DVE
                            act_t += ACT_COST(1024)
                            nc.scalar.activation(pf, stile[:], Exp)
                            dve_t += 600.0
                            nc.vector.tensor_mul(pf, pf, mask01_t[:, r, :])
                        elif r >= 2:
                            # small diagonal blocks: one fused DVE op
                            # (Schraudolph exp + mask bias, u16 saturation)
                            dve_t += DVE_COST(1024)
                            nc.vector.scalar_tensor_tensor(
                                out=pt[:],
                                in0=stile[:],
                                scalar=SCH_A,
                                in1=maskB_t[:, r - 2, :],
                                op0=Mult,
                                op1=Add,
                            )
                        else:
                            ca, cd = ACT_COST(1024), DVE_COST(1024)
                            use_act = act_t + ca <= dve_t + cd
                            # bound same-engine runs at 2 so the two
                            # in-flight score tiles are never serialized
                            # behind a single engine
                            if len(last_eng) >= 2 and last_eng[-1] == last_eng[-2]:
                                use_act = not last_eng[-1]
                            last_eng.append(use_act)
                            if use_act:
                                act_t += ca
                                nc.scalar.activation(pf, stile[:], Exp)
                            else:
                                dve_t += cd
                                nc.vector.tensor_scalar(
                                    pt[:], stile[:], SCH_A, SCH_B, Mult, Add
                                )
                        pblks[i] = pt
                        nissued += 1
                        # the previous group's broadcast matmuls + final muls
                        # land here: same (64,128) PE tile mode as the S
                        # stream, and their DVE inputs are ready by now
                        if nissued == 3 and pending:
                            for fn in pending:
                                fn()
                            pending = []
                    if pending:
                        for fn in pending:
                            fn()
                        pending = []

                    rcf = rcf_ring[nidx % 4]
                    nidx += 1
                    pvp = ps_pv.tile([65, 1024], F32, tag="pv", name="pvp")
                    for hh in range(2):
                        for n, i in enumerate(order):
                            r = i - 4 * J
                            w = 512 if r < 0 else 512 - 128 * r
                            co = hh * 512 + (0 if r < 0 else 128 * r)
                            nc.tensor.matmul(
                                pvp[:, co : co + w],
                                vb4[:, i, 2 * pr + hh, :],
                                pblks[i][:, hh * 512 : hh * 512 + w].bitcast(F16),
                                start=(n == 0),
                                stop=(n == nblk - 1),
                            )
                        if hh == 0:
                            # h0's denominator row is final: stage its copy
                            # so it overlaps the h1 PV burst on the DVE
                            nc.vector.tensor_copy(
                                rcf[64:65, :], pvp[64:65, 0:512]
                            )

                    # 1/den for both heads: f16 bit-trick seed + one Newton
                    # step on rcf rows 64 (h0) and 96 (h1); the ops run over
                    # the whole [64:128] row block (32-aligned starts) — the
                    # other rows turn into finite garbage that the zero rows
                    # of the broadcast stationary annihilate
                    nc.vector.tensor_copy(rcf[96:97, :], pvp[64:65, 512:1024])
                    y0 = small.tile([128, 512], I16, tag="y0", name="y0")
                    nc.vector.tensor_scalar(
                        y0[64:128, :], rcf[64:128, :].bitcast(I16),
                        -1.0, RCP_K, Mult, Add,
                    )
                    t1 = small.tile([128, 512], F16, tag="t1", name="t1")
                    nc.vector.tensor_mul(
                        t1[64:128, :], rcf[64:128, :], y0[64:128, :].bitcast(F16)
                    )
                    nc.vector.tensor_scalar(
                        t1[64:128, :], t1[64:128, :], -1.0, 2.0, Mult, Add
                    )
                    nc.vector.tensor_mul(
                        rcf[64:128, :], y0[64:128, :].bitcast(F16), t1[64:128, :]
                    )
                    dve_t += DVE_NORM_EXTRA

                    def norm(pvp=pvp, sq=sq, pr=pr, rcf=rcf):
                        for hh, blk in ((0, oneblk_t), (1, oneblk96_t)):
                            o = hh * 64
                            bcp = ps_bc.tile(
                                [128, 512], F32, tag="bc", name="bcp"
                            )
                            nc.tensor.matmul(
                                bcp[:],
                                blk[64:128, :],
                                rcf[64:128, :],
                                start=True,
                                stop=True,
                                tile_position=(64, 0),
                            )
                            bcs = small.tile(
                                [64, 512], F32, tag="bcs", name="bcs"
                            )
                            nc.vector.tensor_copy(bcs[:], bcp[0:64, :])
                            nc.vector.tensor_mul(
                                attnT[pr][o : o + 64, sq],
                                pvp[0:64, hh * 512 : (hh + 1) * 512],
                                bcs[:],
                            )

                    pending.append(norm)
                    if pr == 0 and J >= 1:
                        emit_proj(J - 1)
            for fn in pending:
                fn()
            emit_proj(3)

    nc.compile()
    return nc


def _col_perm(g):
    """Per-core W_attn column permutation: [q0..q3 | k0..k3 | v0..v3]."""
    cols = []
    for t in range(3):          # q, k, v
        for h in range(HPC):
            base = (4 * g + h) * 3 * HD + t * HD
            cols.append(np.arange(base, base + HD))
    return np.concatenate(cols)


def kernel(hidden_states, W_attn, b_attn, W_proj, b_proj):
    hidden_states = np.asarray(hidden_states, np.float32)
    W_attn = np.asarray(W_attn, np.float32)
    b_attn = np.asarray(b_attn, np.float32)
    W_proj = np.asarray(W_proj, np.float32)
    b_proj = np.asarray(b_proj, np.float32)

    if "nc" not in _cache:
        _cache["nc"] = _build()
    nc = _cache["nc"]

    # q columns (first 256 of the permuted layout) have scale 1/8 folded into
    # the PSUM->SBUF copy; bias is added after the scale, so pre-scale it.
    bias_scale = np.ones(2 * CP, np.float32)
    bias_scale[:CP] = 0.125

    in_maps = []
    for c in range(8):
        b, g = divmod(c, 4)
        perm = _col_perm(g)
        wa = np.ascontiguousarray(W_attn[:, perm])
        ba = np.ascontiguousarray(
            (b_attn[perm][: 2 * CP] * bias_scale).astype(np.float32).reshape(4, 128).T
        )
        bv = b_attn[perm][2 * CP :].astype(np.float16)
        wp = np.ascontiguousarray(W_proj[g * CP : (g + 1) * CP, :])
        xT = np.ascontiguousarray(hidden_states[b].T).astype(np.float16)
        in_maps.append(
            {
                "x": xT.reshape(NK, 128, S),
                "wa": wa.astype(np.float16).reshape(NK, 128, CW),
                "ba": ba.reshape(1, 128, 4),
                "bv": bv.reshape(1, 1, CP),
                "wp": wp.astype(np.float16).reshape(2, 128, E),
            }
        )

    global _last_in_maps
    _last_in_maps = in_maps
    res = run_bass_kernel_spmd(nc, in_maps, list(range(8)))

    out = np.zeros((B, S, E), np.float32)
    for c in range(8):
        b = c // 4
        out[b] += res.results[c]["out_t"].reshape(E, S).astype(np.float32).T
    out += b_proj
    return out


# revision 54
# speedup vs baseline: 1.0286x; 1.0012x over previous
"""GPT2 attention (B=2, S=2048, E=1024, H=16, interleaved QKV) on 8 trn2 NeuronCores.

Sharding: core c = 4*b + g handles batch b = c//4 and head group g = c%4
(heads 4g..4g+3): Megatron column-split of W_attn / row-split of W_proj,
data-parallel over batch. Host sums the 4 partial projection outputs per batch.

Design (throughput-oriented; baseline 333us -> 213us):
  - X^T is pre-transposed on the host; no PE transposes at all.
  - qk^T = W^T X^T (features on partitions); V computed directly in
    [token, dim] layout via x-stationary matmuls (no V transpose).
  - Scores S^T[sk,sq] per head with 64-deep contraction run 2-way
    concurrent on the two PE row-tiles (tile_position (0,0)/(64,0)),
    one head per half-array; both heads share one [128,1024] score tile
    so every softmax exp call covers two blocks. Diagonal score blocks
    only compute their valid (lower-trapezoid) width.
  - softmax exp is split between ACT (native Exp) and DVE (Schraudolph
    exponent-stuffing: round(x*a+b) as uint16, bitcast to f16) via a
    run-bounded load balancer. Large diagonal blocks (r=0,1) use exact
    ACT exp + a DVE 0/1 mask multiply; small ones (r=2,3) use a single
    fused DVE op whose mask-bias operand drives masked entries negative
    so the u16 convert saturates them to exactly 0.
  - PV appends a ones column to V so PSUM row 64 = softmax denominators;
    1/den via an f16 bit-trick seed + one Newton step on the denominator
    rows, broadcast with a rank-1 PE matmul kept in the same (64,128)
    tile mode as the score stream, then one DVE multiply. The broadcast
    matmuls are deferred into the next group's score stream so their DVE
    inputs are ready when the PE reaches them.
  - the output projection is interleaved into the attention stream (one
    sq-chunk right after each PV burst, same 128x128 tile mode), its
    PSUM shared with the broadcast pool; evacuation is f16 alternating
    ACT/DVE, halving the output DMA.
  - input DMAs are spread over the sync/scalar/gpsimd queues (each
    dma_start costs ~650ns of queue issue time) with wa[0]/xT[0] first.
"""
import numpy as np

import concourse.bass as bass
import concourse.bacc as bacc
import concourse.tile as tile
from concourse import mybir
from concourse.bass_utils import run_bass_kernel_spmd

F32 = mybir.dt.float32
F16 = mybir.dt.float16
I16 = mybir.dt.int16
U16 = mybir.dt.uint16

B, S, E, H = 2, 2048, 1024, 16
HD = E // H            # 64
HPC = 4                # heads per core
CW = HPC * 3 * HD      # 768: W_attn cols per core
CP = HPC * HD          # 256: W_proj rows per core
NK = E // 128          # 8 contraction chunks over E
NSQ = S // 512         # 4 sq chunks of 512
NSK = S // 128         # 16 sk chunks of 128

# Schraudolph exp: exp(x) ~= bitcast_f16(round(x * SCH_A + SCH_B)); the u16
# output convert saturates negatives to 0, so masked entries (biased by
# -60000 via the fused mask operand) become exactly +0.0
SCH_A = 1024.0 / float(np.log(2.0))
SCH_B = 15312.0  # 15360 - 48, f16-exact so the mask constant tiles match
MASKED = -60000.0
# f16 reciprocal seed: 1/d ~= bitcast_f16(RCP_K - bits_f16(d)), then one
# Newton step y1 = y0 * (2 - d*y0); max rel err ~3e-3
RCP_K = 30620.0

# dispatch cost model (ns) for the exp of a [128, n]-column pair tile
ACT_COST = lambda n: (n + 352) / 1.2
DVE_COST = lambda n: 0.52 * n + 300.0
DVE_NORM_EXTRA = 5000.0  # den copies + newton + bcs + muls per (pair, J)

_cache = {}
_last_in_maps = None


def _build():
    from contextlib import ExitStack

    nc = bacc.Bacc("TRN2", target_bir_lowering=False, debug=False, num_devices=8)

    x_d = nc.dram_tensor("x", [NK, 128, S], F16, kind="ExternalInput").ap()
    wa_d = nc.dram_tensor("wa", [NK, 128, CW], F16, kind="ExternalInput").ap()
    ba_d = nc.dram_tensor("ba", [1, 128, 4], F32, kind="ExternalInput").ap()
    bv_d = nc.dram_tensor("bv", [1, 1, CP], F16, kind="ExternalInput").ap()
    wp_d = nc.dram_tensor("wp", [2, 128, E], F16, kind="ExternalInput").ap()
    out_d = nc.dram_tensor("out_t", [8, 128, S], F16, kind="ExternalOutput").ap()

    # diagonal-block masks in pair layout [h0: 0..w | gap | h1: 512..512+w |
    # gap], reduced coords g (block col), keep where g >= p.
    # r=0,1 (large blocks): 0/1 multiply masks applied after an exact ACT exp.
    # r=2,3 (small blocks): fused bias masks for the DVE Schraudolph exp
    # (keep -> +SCH_B, masked -> -60000 so the u16 convert saturates to 0).
    gi = np.arange(512)[None, :]
    pi = np.arange(128)[:, None]
    mask01_d = {}
    maskB_d = {}
    for r in range(4):
        w = 512 - 128 * r
        keep = (gi < w) & (gi >= pi)
        if r < 2:
            half = np.zeros((128, 512), np.float16)
            half[keep] = 1.0
            mask01_d[r] = nc.inline_tensor(
                np.concatenate([half, half], axis=1), name=f"mask01_{r}"
            )
        else:
            half = np.full((128, 512), MASKED, np.float16)
            half[keep] = SCH_B
            maskB_d[r] = nc.inline_tensor(
                np.concatenate([half, half], axis=1), name=f"maskB{r}"
            )
    ones1_d = nc.inline_tensor(np.ones((1, 128), np.float16), name="ones1")
    # rank-1 broadcast stationaries: row 64 (and 65) = ones, used in the
    # (64,128) tile mode so they do not break the score-stream mode
    oneblk_np = np.zeros((128, 128), np.float16)
    oneblk_np[64, :] = 1.0
    oneblk96_np = np.zeros((128, 128), np.float16)
    oneblk96_np[96, :] = 1.0
    oneblk_d = nc.inline_tensor(oneblk_np, name="oneblk")
    oneblk96_d = nc.inline_tensor(oneblk96_np, name="oneblk96")

    Exp = mybir.ActivationFunctionType.Exp
    Ident = mybir.ActivationFunctionType.Identity
    Mult = mybir.AluOpType.mult
    Add = mybir.AluOpType.add

    with tile.TileContext(nc) as tc, ExitStack() as top:
        consts = top.enter_context(tc.tile_pool(name="consts", bufs=1))
        qk_pool = top.enter_context(tc.tile_pool(name="qkT", bufs=1))
        at_pool = top.enter_context(tc.tile_pool(name="attnT", bufs=1))
        wp_pool = top.enter_context(tc.tile_pool(name="wp", bufs=1))
        vb_pool = top.enter_context(tc.tile_pool(name="vb", bufs=1))
        xTp = top.enter_context(tc.tile_pool(name="xT", bufs=1))

        # ---- input DMAs -------------------------------------------------
        # each dma_start costs ~650ns of queue issue time, so spread the
        # loads over four queues and put the critical first inputs (wa[0],
        # xT[0]) at the head of their queues
        xT = [xTp.tile([128, S], F16, tag=f"xT{k}", name=f"xT{k}") for k in range(NK)]
        wa_t = consts.tile([128, NK, CW], F16)
        dq = [nc.sync, nc.scalar, nc.gpsimd]
        for k in range(NK):
            nc.gpsimd.dma_start(out=wa_t[:, k, :], in_=wa_d[k])
            dq[k % 3].dma_start(out=xT[k][:], in_=x_d[k])
        ba_t = consts.tile([128, 4], F32)
        nc.scalar.dma_start(out=ba_t[:], in_=ba_d[0])
        ones1_t = consts.tile([1, 128], F16)
        nc.scalar.dma_start(out=ones1_t[:], in_=ones1_d.ap())
        bv_t = consts.tile([1, CP], F16)
        nc.scalar.dma_start(out=bv_t[:], in_=bv_d[0])
        wp_t = wp_pool.tile([128, 2, E], F16)
        for cc in range(2):
            nc.sync.dma_start(out=wp_t[:, cc, :], in_=wp_d[cc])
        mask01_t = consts.tile([128, 2, 1024], F16)
        for r in range(2):
            nc.sync.dma_start(out=mask01_t[:, r, :], in_=mask01_d[r].ap())
        maskB_t = consts.tile([128, 2, 1024], F16)
        for r in range(2, 4):
            nc.sync.dma_start(out=maskB_t[:, r - 2, :], in_=maskB_d[r].ap())

        qkvT = [
            qk_pool.tile([128, S], F16, tag=f"qkT{cc}", name=f"qkT{cc}")
            for cc in range(4)
        ]
        attnT = [
            at_pool.tile([128, S], F16, tag=f"attnT{c}", name=f"attnT{c}")
            for c in range(2)
        ]
        # V with a ones column appended: [sk-chunk partitions, i, head, 65]
        vb4 = vb_pool.tile([128, NSK, HPC, 65], F16)
        nc.gpsimd.memset(vb4[:, :, :, 64:65], 1.0)

        # ---- phase 1: qk^T = W^T X^T, V = X Wv --------------------------
        with (
            tc.tile_pool(name="ps_mm", bufs=4, space="PSUM") as ps_mm,
            tc.tile_pool(name="ps_v", bufs=3, space="PSUM") as ps_v,
            tc.tile_pool(name="ps_b", bufs=1, space="PSUM") as ps_b,
        ):
            bvbc = consts.tile([128, HPC, 64], F32)

            for cc in range(4):
                if cc == 1:
                    # v-bias broadcast (32-row tile mode), tucked in after
                    # cc=0 so the kernel's first matmul is not gated on the
                    # small-constant DMAs
                    psb = ps_b.tile([128, HPC, 64], F32, tag="pvb", name="pvb")
                    nc.tensor.matmul(
                        psb[:], ones1_t[0:1, :], bv_t[0:1, :], start=True, stop=True
                    )
                    nc.vector.tensor_copy(bvbc[:], psb[:])
                pss = [
                    ps_mm.tile([128, 512], F32, tag="mm", name="mm_ps")
                    for _ in range(4)
                ]
                for k in range(NK):
                    lhs = wa_t[:, k, cc * 128 : (cc + 1) * 128]
                    for rc in range(4):
                        nc.tensor.matmul(
                            pss[rc][:],
                            lhs,
                            xT[k][:, rc * 512 : (rc + 1) * 512],
                            start=(k == 0),
                            stop=(k == NK - 1),
                        )
                for rc in range(4):
                    nc.scalar.activation(
                        qkvT[cc][:, rc * 512 : (rc + 1) * 512],
                        pss[rc][:],
                        Ident,
                        bias=ba_t[:, cc : cc + 1],
                        scale=0.125 if cc < 2 else 1.0,
                    )

            for i in range(NSK):
                psv = ps_v.tile([128, HPC, 64], F32, tag="pv", name="pv_ps")
                for k in range(NK):
                    nc.tensor.matmul(
                        psv[:],
                        xT[k][:, i * 128 : (i + 1) * 128],
                        wa_t[:, k, 512:768],
                        start=(k == 0),
                        stop=(k == NK - 1),
                    )
                nc.vector.tensor_add(vb4[:, i, :, 0:64], psv[:], bvbc[:])

        # ---- phase 2: per-head-pair attention ---------------------------
        # Both heads of a pair share one [128,1024] score tile (one bank per
        # head), so every exp call covers two blocks. Diagonal blocks use the
        # DVE fused op (x*A + maskB) with u16 saturation; off-diagonal blocks
        # are load-balanced between ACT Exp and DVE Schraudolph.
        with (
            tc.tile_pool(name="pp", bufs=18) as p_pool,
            tc.tile_pool(name="sm", bufs=4) as small,
            tc.tile_pool(name="rcf", bufs=1) as rcf_pool,
            tc.tile_pool(name="ps_s", bufs=2, space="PSUM") as ps_s,
            tc.tile_pool(name="ps_pv", bufs=1, space="PSUM") as ps_pv,
            tc.tile_pool(name="ps_bc", bufs=2, space="PSUM") as ps_bc,
            tc.tile_pool(name="ob", bufs=6) as ob_pool,
        ):
            oneblk_t = consts.tile([128, 128], F16)
            nc.gpsimd.dma_start(out=oneblk_t[:], in_=oneblk_d.ap())
            oneblk96_t = consts.tile([128, 128], F16)
            nc.gpsimd.dma_start(out=oneblk96_t[:], in_=oneblk96_d.ap())
            # rcp rows ring: rows 64/96 carry 1/den (h0/h1) as f16 for the
            # rank-1 broadcast matmuls (engine partition starts must be
            # 32-aligned); other rows zeroed once (NaN guard)
            # guard rows start at 1.0: the newton ops iterate y <- ~1/y on
            # them, for which 1.0 is a stable bounded fixed point (0.0 would
            # blow up to inf and 0*inf would NaN the broadcast matmul)
            rcf_ring = []
            for n in range(4):
                t = rcf_pool.tile([128, 512], F16, tag=f"rcf{n}", name=f"rcf{n}")
                nc.vector.memset(t[64:128, :], 1.0)
                rcf_ring.append(t)

            act_t = 0.0
            dve_t = 0.0
            last_eng = []
            pending = []
            nidx = 0

            def emit_proj(Jq):
                # projection for sq chunk Jq, interleaved into the attention
                # stream right after a PV burst (same 128x128 tile mode);
                # shares the ps_bc pool's banks
                for eo in range(8):
                    pp = ps_bc.tile([128, 512], F32, tag="bc", name="prj")
                    for cc in range(2):
                        nc.tensor.matmul(
                            pp[:],
                            wp_t[:, cc, eo * 128 : (eo + 1) * 128],
                            attnT[cc][:, Jq * 512 : (Jq + 1) * 512],
                            start=(cc == 0),
                            stop=(cc == 1),
                        )
                    ob = ob_pool.tile([128, 512], F16, tag="ob", name="ob")
                    if eo % 2 == 0:
                        nc.scalar.copy(ob[:], pp[:])
                    else:
                        nc.vector.tensor_copy(ob[:], pp[:])
                    # all output triggers on the (otherwise idle) sync queue:
                    # a trigger + its semaphore wait on the ACT queue blocks
                    # the strict FIFO ahead of pending exp work
                    nc.sync.dma_start(
                        out=out_d[eo][:, Jq * 512 : (Jq + 1) * 512], in_=ob[:]
                    )

            for J in range(NSQ):
                for pr in range(2):
                    qT = qkvT[pr]
                    kT = qkvT[2 + pr]
                    nblk = 4 * J + 4
                    sq = bass.ts(J, 512)
                    # block order: diagonal r=0..3 first, then off-diagonal
                    order = [4 * J + r for r in range(4)] + list(range(4 * J))
                    pblks = {}
                    nissued = 0
                    for i in order:
                        r = i - 4 * J
                        w = 512 if r < 0 else 512 - 128 * r
                        sqo = J * 512 + (0 if r < 0 else 128 * r)
                        stile = ps_s.tile([128, 1024], F32, tag="s", name="sps")
                        for hh in range(2):
                            o = hh * 64
                            nc.tensor.matmul(
                                stile[:, hh * 512 : hh * 512 + w],
                                kT[o : o + 64, i * 128 : (i + 1) * 128],
                                qT[o : o + 64, sqo : sqo + w],
                                start=True,
                                stop=True,
                                tile_position=(o, 0),
                            )
                        pt = p_pool.tile([128, 1024], U16, tag="p", name="p")
                        pf = pt[:].bitcast(F16)
                        if r >= 0 and r < 2:
                            # large diagonal blocks: exact exp on ACT, then a
                            # fast 0/1 mask multiply on DVE
                            act_t += ACT_COST(1024)
                            nc.scalar.activation(pf, stile[:], Exp)
                            dve_t += 600.0
                            nc.vector.tensor_mul(pf, pf, mask01_t[:, r, :])
                        elif r >= 2:
                            # small diagonal blocks: one fused DVE op
                            # (Schraudolph exp + mask bias, u16 saturation)
                            dve_t += DVE_COST(1024)
                            nc.vector.scalar_tensor_tensor(
                                out=pt[:],
                                in0=stile[:],
                                scalar=SCH_A,
                                in1=maskB_t[:, r - 2, :],
                                op0=Mult,
                                op1=Add,
                            )
                        else:
                            ca, cd = ACT_COST(1024), DVE_COST(1024)
                            use_act = act_t + ca <= dve_t + cd
                            # bound same-engine runs at 2 so the two
                            # in-flight score tiles are never serialized
                            # behind a single engine
                            if len(last_eng) >= 2 and last_eng[-1] == last_eng[-2]:
                                use_act = not last_eng[-1]
                            last_eng.append(use_act)
                            if use_act:
                                act_t += ca
                                nc.scalar.activation(pf, stile[:], Exp)
                            else:
                                dve_t += cd
                                nc.vector.tensor_scalar(
                                    pt[:], stile[:], SCH_A, SCH_B, Mult, Add
                                )
                        pblks[i] = pt
                        nissued += 1
                        # the previous group's broadcast matmuls + final muls
                        # land here: same (64,128) PE tile mode as the S
                        # stream, and their DVE inputs are ready by now
                        if nissued == 3 and pending:
                            for fn in pending:
                                fn()
                            pending = []
                    if pending:
                        for fn in pending:
                            fn()
                        pending = []

                    rcf = rcf_ring[nidx % 4]
                    nidx += 1
                    pvp = ps_pv.tile([65, 1024], F32, tag="pv", name="pvp")
                    for hh in range(2):
                        for n, i in enumerate(order):
                            r = i - 4 * J
                            w = 512 if r < 0 else 512 - 128 * r
                            co = hh * 512 + (0 if r < 0 else 128 * r)
                            nc.tensor.matmul(
                                pvp[:, co : co + w],
                                vb4[:, i, 2 * pr + hh, :],
                                pblks[i][:, hh * 512 : hh * 512 + w].bitcast(F16),
                                start=(n == 0),
                                stop=(n == nblk - 1),
                            )
                        if hh == 0:
                            # h0's denominator row is final: stage its copy
                            # so it overlaps the h1 PV burst on the DVE
                            nc.vector.tensor_copy(
                                rcf[64:65, :], pvp[64:65, 0:512]
                            )

                    # 1/den for both heads: f16 bit-trick seed + one Newton
                    # step on rcf rows 64 (h0) and 96 (h1); the ops run over
                    # the whole [64:128] row block (32-aligned starts) — the
                    # other rows turn into finite garbage that the zero rows
                    # of the broadcast stationary annihilate
                    nc.vector.tensor_copy(rcf[96:97, :], pvp[64:65, 512:1024])
                    y0 = small.tile([128, 512], I16, tag="y0", name="y0")
                    nc.vector.tensor_scalar(
                        y0[64:128, :], rcf[64:128, :].bitcast(I16),
                        -1.0, RCP_K, Mult, Add,
                    )
                    t1 = small.tile([128, 512], F16, tag="t1", name="t1")
                    nc.vector.tensor_mul(
                        t1[64:128, :], rcf[64:128, :], y0[64:128, :].bitcast(F16)
                    )
                    nc.vector.tensor_scalar(
                        t1[64:128, :], t1[64:128, :], -1.0, 2.0, Mult, Add
                    )
                    nc.vector.tensor_mul(
                        rcf[64:128, :], y0[64:128, :].bitcast(F16), t1[64:128, :]
                    )
                    dve_t += DVE_NORM_EXTRA

                    def norm(pvp=pvp, sq=sq, pr=pr, rcf=rcf):
                        for hh, blk in ((0, oneblk_t), (1, oneblk96_t)):
                            o = hh * 64
                            bcp = ps_bc.tile(
                                [128, 512], F32, tag="bc", name="bcp"
                            )
                            nc.tensor.matmul(
                                bcp[:],
                                blk[64:128, :],
                                rcf[64:128, :],
                                start=True,
                                stop=True,
                                tile_position=(64, 0),
                            )
                            bcs = small.tile(
                                [64, 512], F32, tag="bcs", name="bcs"
                            )
                            nc.vector.tensor_copy(bcs[:], bcp[0:64, :])
                            nc.vector.tensor_mul(
                                attnT[pr][o : o + 64, sq],
                                pvp[0:64, hh * 512 : (hh + 1) * 512],
                                bcs[:],
                            )

                    pending.append(norm)
                    if pr == 0 and J >= 1:
                        emit_proj(J - 1)
            for fn in pending:
                fn()
            emit_proj(3)

    nc.compile()
    return nc


def _col_perm(g):
    """Per-core W_attn column permutation: [q0..q3 | k0..k3 | v0..v3]."""
    cols = []
    for t in range(3):          # q, k, v
        for h in range(HPC):
            base = (4 * g + h) * 3 * HD + t * HD
            cols.append(np.arange(base, base + HD))
    return np.concatenate(cols)


def kernel(hidden_states, W_attn, b_attn, W_proj, b_proj):
    hidden_states = np.asarray(hidden_states, np.float32)
    W_attn = np.asarray(W_attn, np.float32)
    b_attn = np.asarray(b_attn, np.float32)
    W_proj = np.asarray(W_proj, np.float32)
    b_proj = np.asarray(b_proj, np.float32)

    if "nc" not in _cache:
        _cache["nc"] = _build()
    nc = _cache["nc"]

    # q columns (first 256 of the permuted layout) have scale 1/8 folded into
    # the PSUM->SBUF copy; bias is added after the scale, so pre-scale it.
    bias_scale = np.ones(2 * CP, np.float32)
    bias_scale[:CP] = 0.125

    in_maps = []
    for c in range(8):
        b, g = divmod(c, 4)
        perm = _col_perm(g)
        wa = np.ascontiguousarray(W_attn[:, perm])
        ba = np.ascontiguousarray(
            (b_attn[perm][: 2 * CP] * bias_scale).astype(np.float32).reshape(4, 128).T
        )
        bv = b_attn[perm][2 * CP :].astype(np.float16)
        wp = np.ascontiguousarray(W_proj[g * CP : (g + 1) * CP, :])
        xT = np.ascontiguousarray(hidden_states[b].T).astype(np.float16)
        in_maps.append(
            {
                "x": xT.reshape(NK, 128, S),
                "wa": wa.astype(np.float16).reshape(NK, 128, CW),
                "ba": ba.reshape(1, 128, 4),
                "bv": bv.reshape(1, 1, CP),
                "wp": wp.astype(np.float16).reshape(2, 128, E),
            }
        )

    global _last_in_maps
    _last_in_maps = in_maps
    res = run_bass_kernel_spmd(nc, in_maps, list(range(8)))

    out = np.zeros((B, S, E), np.float32)
    for c in range(8):
        b = c // 4
        out[b] += res.results[c]["out_t"].reshape(E, S).astype(np.float32).T
    out += b_proj
    return out
